# revision 8
# baseline (speedup 1.0000x reference)
"""EntmaxAttention TRN2 kernel.

Full inputs -> full outputs. Internally shards the batch (B=8) across 8
NeuronCores (data parallel, no collectives). Each core runs the complete
attention pipeline for its batch element:

  Q^T/K^T/V projections (fp32r matmuls) -> per-head scores (fp32r) ->
  entmax-1.5 via 5 support-set quadratic-solve iterations (exact solve of
  sum(relu(x-tau)^2)=1 on the current support; converges to the same fp32
  fixpoint as the reference's 50-step bisection) -> attn normalize ->
  PE-transpose attn -> ctx^T (fp32r) -> output projection (fp32r).

Engine split per entmax iteration (per [128,1024] tile):
  DVE : w = max(xm, tau)  (accum -> sum w  => m1 = sum w - 1024*tau)
  ACT : p = (tau - w)^2   (accum -> m2)
  DVE : count = sum(xm > tau)
Per-row state for all 8 tiles of a head lives in one [128, *] tile so the
scalar updates are single instructions per head-iteration.
"""

import sys
import numpy as np
from contextlib import ExitStack

if "/opt/trn_rl_repo" not in sys.path:
    sys.path.insert(0, "/opt/trn_rl_repo")

import concourse.bacc as bacc
import concourse.tile as tile
import concourse.mybir as mybir
from concourse.bass_utils import run_bass_kernel_spmd
from concourse.masks import make_identity

f32 = mybir.dt.float32
f32r = mybir.dt.float32r
Alu = mybir.AluOpType
Act = mybir.ActivationFunctionType
AxX = mybir.AxisListType.X

B, N, C, H = 8, 1024, 1024, 8
D = C // H          # 128
P = 128             # partitions
NT = N // P         # 8 row tiles per head
CH = C // P         # 8 contraction chunks
SCALE = float(np.float32(0.5 / np.sqrt(D)))   # (alpha-1) / sqrt(D)
N_ITERS = 5

_CACHE = {}


def _build():
    nc = bacc.Bacc("TRN2", target_bir_lowering=False, debug=False,
                   enable_asserts=False, num_devices=8)

    x_d = nc.dram_tensor("x", [N, C], f32, kind="ExternalInput").ap()
    w_d = {}
    b_d = {}
    for nm in ("q", "k", "v", "o"):
        w_d[nm] = nc.dram_tensor(f"W{nm}", [C, C], f32, kind="ExternalInput").ap()
        b_d[nm] = nc.dram_tensor(f"b{nm}", [C], f32, kind="ExternalInput").ap()
    out_d = nc.dram_tensor("out", [N, C], f32, kind="ExternalOutput").ap()
    attn_d = nc.dram_tensor("attn", [H, N, N], f32, kind="ExternalOutput").ap()

    # DRAM scratch: Q^T/K^T (chunk == head), V (natural), ctx^T per head
    qT_d = nc.dram_tensor("qT_scr", [H, P, N], f32).ap()
    kT_d = nc.dram_tensor("kT_scr", [H, P, N], f32).ap()
    v_d = nc.dram_tensor("v_scr", [N, C], f32).ap()
    cx_d = nc.dram_tensor("cx_scr", [H, P, N], f32).ap()

    ones_host = nc.inline_tensor(np.ones((1, P), dtype=np.float32), name="ones_c")

    with tile.TileContext(nc) as tc, ExitStack() as octx:
        consts = octx.enter_context(tc.tile_pool(name="consts", bufs=1))
        ident = consts.tile([P, P], f32)
        make_identity(nc, ident[:])
        ones_sb = consts.tile([1, P], f32r)
        nc.sync.dma_start(out=ones_sb[:], in_=ones_host.ap().bitcast(f32r))
        # bias rows ([1, C]) for the ones-trick matmuls (bv, bo)
        bias_rows = consts.tile([1, 2 * C], f32r)
        nc.sync.dma_start(out=bias_rows[:, 0:C],
                          in_=b_d["v"].rearrange("(a c) -> a c", a=1).bitcast(f32r))
        nc.sync.dma_start(out=bias_rows[:, C:2 * C],
                          in_=b_d["o"].rearrange("(a c) -> a c", a=1).bitcast(f32r))
        # bias columns ([P, CH]) for Q^T / K^T evictions
        bias_cols = consts.tile([P, 2 * CH], f32)
        nc.sync.dma_start(out=bias_cols[:, 0:CH],
                          in_=b_d["q"].rearrange("(c p) -> p c", p=P))
        nc.sync.dma_start(out=bias_cols[:, CH:2 * CH],
                          in_=b_d["k"].rearrange("(c p) -> p c", p=P))

        # ---------------- Phase 0+1: x^T, projections ----------------
        with ExitStack() as ctx:
            xpool = ctx.enter_context(tc.tile_pool(name="xsb", bufs=NT))
            xTpool = ctx.enter_context(tc.tile_pool(name="xT", bufs=CH))
            tps = ctx.enter_context(tc.tile_pool(name="tps", bufs=4, space="PSUM"))
            x_sb = []
            for i in range(NT):
                t = xpool.tile([P, C], f32, tag="xsb")
                nc.sync.dma_start(out=t[:], in_=x_d[i * P:(i + 1) * P, :])
                x_sb.append(t)
            xT = [xTpool.tile([P, N], f32r, tag="xT", name=f"xT{ck}")
                  for ck in range(CH)]
            for ck in range(CH):
                for i2 in range(0, NT, 4):
                    pt = tps.tile([P, 4 * P], f32, tag="tp")
                    for j in range(4):
                        i = i2 + j
                        nc.tensor.transpose(pt[:, j * P:(j + 1) * P],
                                            x_sb[i][:, ck * P:(ck + 1) * P],
                                            ident[:])
                    nc.vector.tensor_copy(xT[ck][:, i2 * P:(i2 + 4) * P], pt[:])

            wpool = ctx.enter_context(tc.tile_pool(name="wsb", bufs=2 * CH))
            stage = ctx.enter_context(tc.tile_pool(name="stage", bufs=3))
            pps = ctx.enter_context(tc.tile_pool(name="pps", bufs=4, space="PSUM"))

            for wi, nm in enumerate(("q", "k")):
                w_sb = []
                for ck in range(CH):
                    t = wpool.tile([P, C], f32r, tag="w")
                    nc.sync.dma_start(out=t[:],
                                      in_=w_d[nm][ck * P:(ck + 1) * P, :].bitcast(f32r))
                    w_sb.append(t)
                dst = qT_d if nm == "q" else kT_d
                bcol = bias_cols[:, wi * CH:(wi + 1) * CH]
                for cp in range(CH):
                    acc = stage.tile([P, N], f32, tag="stage")
                    for half in range(2):
                        ps = pps.tile([P, N // 2], f32, tag="pp")
                        for ck in range(CH):
                            nc.tensor.matmul(
                                ps[:],
                                w_sb[ck][:, cp * P:(cp + 1) * P],
                                xT[ck][:, half * 512:(half + 1) * 512],
                                start=(ck == 0), stop=(ck == CH - 1))
                        nc.scalar.activation(acc[:, half * 512:(half + 1) * 512],
                                             ps[:], Act.Identity,
                                             bias=bcol[:, cp:cp + 1], scale=1.0)
                    nc.sync.dma_start(out=dst[cp], in_=acc[:])

            # V = x @ Wv + bv  (natural layout, row chunks)
            w_sb = []
            for ck in range(CH):
                t = wpool.tile([P, C], f32r, tag="w")
                nc.sync.dma_start(out=t[:],
                                  in_=w_d["v"][ck * P:(ck + 1) * P, :].bitcast(f32r))
                w_sb.append(t)
            for mc in range(NT):
                acc = stage.tile([P, C], f32, tag="stage")
                for half in range(2):
                    ps = pps.tile([P, C // 2], f32, tag="pp")
                    for ck in range(CH):
                        nc.tensor.matmul(
                            ps[:],
                            xT[ck][:, mc * P:(mc + 1) * P],
                            w_sb[ck][:, half * 512:(half + 1) * 512],
                            start=(ck == 0), stop=False)
                    nc.tensor.matmul(ps[:], ones_sb[:],
                                     bias_rows[:, half * 512:(half + 1) * 512],
                                     start=False, stop=True)
                    nc.scalar.copy(acc[:, half * 512:(half + 1) * 512], ps[:])
                nc.sync.dma_start(out=v_d[mc * P:(mc + 1) * P, :], in_=acc[:])

        # ---------------- Phase 2: per-head attention ----------------
        with ExitStack() as ctx:
            qkv = ctx.enter_context(tc.tile_pool(name="qkv", bufs=2))
            xmp = ctx.enter_context(tc.tile_pool(name="xm", bufs=11))
            wsc = ctx.enter_context(tc.tile_pool(name="wsc", bufs=4))
            ppool = ctx.enter_context(tc.tile_pool(name="pp", bufs=3))
            apool = ctx.enter_context(tc.tile_pool(name="attn", bufs=NT + 1))
            aT = ctx.enter_context(tc.tile_pool(name="aT", bufs=3))
            stp = ctx.enter_context(tc.tile_pool(name="state", bufs=6))
            cxp = ctx.enter_context(tc.tile_pool(name="cxp", bufs=2))
            sps = ctx.enter_context(tc.tile_pool(name="sps", bufs=2, space="PSUM"))
            tps2 = ctx.enter_context(tc.tile_pool(name="tps2", bufs=2, space="PSUM"))
            cps = ctx.enter_context(tc.tile_pool(name="cps", bufs=2, space="PSUM"))

            G = 4  # tiles per entmax group (half a head)

            def entmax_group(h, half, qh, kh):
                """scores+entmax for tiles [half*G, half*G+G) of head h.
                Returns list of G normalized attn tiles (already DMA'd out)."""
                st = stp.tile([P, 16 * G], f32, tag="st", name=f"st{h}_{half}")
                mx = st[:, 0 * G:1 * G]
                tau = st[:, 1 * G:2 * G]
                sw = st[:, 2 * G:3 * G]
                m2 = st[:, 3 * G:4 * G]
                kc = st[:, 4 * G:5 * G]
                m1 = st[:, 5 * G:6 * G]
                t2 = st[:, 6 * G:7 * G]
                m1s = st[:, 7 * G:8 * G]
                disc = st[:, 8 * G:9 * G]
                sq = st[:, 9 * G:10 * G]
                num = st[:, 10 * G:11 * G]
                rk = st[:, 11 * G:12 * G]
                dlt = st[:, 12 * G:13 * G]
                sS = st[:, 13 * G:14 * G]
                rS = st[:, 14 * G:15 * G]

                xm = []
                for g in range(G):
                    nt = half * G + g
                    t = xmp.tile([P, N], f32, tag="xm", name=f"xm{h}_{nt}")
                    ps = sps.tile([P, N], f32, tag="sp")
                    for hf in range(2):
                        nc.tensor.matmul(ps[:, hf * 512:(hf + 1) * 512],
                                         qh[:, nt * P:(nt + 1) * P],
                                         kh[:, hf * 512:(hf + 1) * 512],
                                         start=True, stop=True)
                    # fused evict: xm = SCALE*psum, accum-max -> mx
                    nc.vector.tensor_scalar(
                        out=t[:], in0=ps[:], scalar1=SCALE, scalar2=None,
                        op0=Alu.mult, op1=Alu.max, accum_out=mx[:, g:g + 1])
                    xm.append(t)

                nc.vector.tensor_scalar(out=tau, in0=mx, scalar1=-1.0, scalar2=None,
                                        op0=Alu.add)

                at_g = []
                for it in range(N_ITERS):
                    last = (it == N_ITERS - 1)
                    for g in range(G):
                        w = wsc.tile([P, N], f32, tag="w", name=f"w{h}_{half}_{g}")
                        nc.vector.tensor_scalar(
                            out=w[:], in0=xm[g][:], scalar1=tau[:, g:g + 1],
                            scalar2=None, op0=Alu.max, op1=Alu.add,
                            accum_out=sw[:, g:g + 1])
                        if last:
                            pt = ppool.tile([P, N], f32, tag="p",
                                            name=f"p{h}_{half}_{g}")
                            acc = sS[:, g:g + 1]
                        else:
                            pt = wsc.tile([P, N], f32, tag="psc",
                                          name=f"psc{h}_{half}_{g}")
                            acc = m2[:, g:g + 1]
                        nc.scalar.activation(pt[:], w[:], Act.Square,
                                             bias=tau[:, g:g + 1], scale=-1.0,
                                             accum_out=acc)
                        if not last:
                            msk = wsc.tile([P, N], f32, tag="psc",
                                           name=f"msk{h}_{half}_{g}")
                            nc.vector.tensor_scalar(
                                out=msk[:], in0=xm[g][:],
                                scalar1=tau[:, g:g + 1], scalar2=None,
                                op0=Alu.is_gt, op1=Alu.add,
                                accum_out=kc[:, g:g + 1])
                        else:
                            nt = half * G + g
                            nc.vector.reciprocal(rS[:, g:g + 1], sS[:, g:g + 1])
                            a = apool.tile([P, N], f32, tag="a",
                                           name=f"a{h}_{nt}")
                            nc.scalar.activation(a[:], pt[:], Act.Copy,
                                                 bias=0.0, scale=rS[:, g:g + 1])
                            nc.sync.dma_start(
                                out=attn_d[h, nt * P:(nt + 1) * P, :], in_=a[:])
                            at_g.append(a)
                    if last:
                        break
                    # tau += (m1 - sqrt(max(m1^2 - kc*(m2-1), 0))) / kc
                    nc.vector.scalar_tensor_tensor(out=m1, in0=tau, scalar=-float(N),
                                                   in1=sw, op0=Alu.mult, op1=Alu.add)
                    nc.vector.scalar_tensor_tensor(out=t2, in0=m2, scalar=-1.0,
                                                   in1=kc, op0=Alu.add, op1=Alu.mult)
                    nc.scalar.activation(m1s, m1, Act.Square, bias=0.0, scale=1.0)
                    nc.vector.scalar_tensor_tensor(out=disc, in0=t2, scalar=-1.0,
                                                   in1=m1s, op0=Alu.mult, op1=Alu.add)
                    nc.vector.tensor_scalar(out=disc, in0=disc, scalar1=0.0,
                                            scalar2=None, op0=Alu.max)
                    nc.scalar.activation(sq, disc, Act.Sqrt, bias=0.0, scale=1.0)
                    nc.vector.scalar_tensor_tensor(out=num, in0=sq, scalar=-1.0,
                                                   in1=m1, op0=Alu.mult, op1=Alu.add)
                    nc.vector.reciprocal(rk, kc)
                    nc.vector.scalar_tensor_tensor(out=dlt, in0=num, scalar=1.0,
                                                   in1=rk, op0=Alu.mult, op1=Alu.mult)
                    nc.vector.scalar_tensor_tensor(out=tau, in0=dlt, scalar=1.0,
                                                   in1=tau, op0=Alu.mult, op1=Alu.add)
                return at_g

            for h in range(H):
                qh = qkv.tile([P, N], f32r, tag="qh", name=f"qh{h}")
                kh = qkv.tile([P, N], f32r, tag="kh", name=f"kh{h}")
                vh = qkv.tile([P, C], f32r, tag="vh", name=f"vh{h}")
                nc.sync.dma_start(out=qh[:], in_=qT_d[h].bitcast(f32r))
                nc.sync.dma_start(out=kh[:], in_=kT_d[h].bitcast(f32r))
                nc.sync.dma_start(
                    out=vh[:].rearrange("p (c d) -> p c d", c=CH),
                    in_=v_d[:, h * D:(h + 1) * D]
                    .rearrange("(c p) d -> p c d", p=P).bitcast(f32r))

                at = entmax_group(h, 0, qh, kh) + entmax_group(h, 1, qh, kh)

                # transpose attn -> attnT chunks; ctx^T accumulation
                cT = cxp.tile([P, N], f32r, tag="cT", name=f"cT{h}")
                cps0 = cps.tile([P, N // 2], f32, tag="cp", name=f"cps0_{h}")
                cps1 = cps.tile([P, N // 2], f32, tag="cp", name=f"cps1_{h}")
                for mc in range(NT):
                    aT_mc = aT.tile([P, N], f32r, tag="aT", name=f"aT{h}_{mc}")
                    for nt2 in range(0, NT, 4):
                        pt = tps2.tile([P, 4 * P], f32, tag="tp2")
                        for j in range(4):
                            nt = nt2 + j
                            nc.tensor.transpose(pt[:, j * P:(j + 1) * P],
                                                at[nt][:, mc * P:(mc + 1) * P],
                                                ident[:])
                        nc.vector.tensor_copy(aT_mc[:, nt2 * P:(nt2 + 4) * P], pt[:])
                    nc.tensor.matmul(cps0[:], vh[:, mc * P:(mc + 1) * P],
                                     aT_mc[:, 0:512],
                                     start=(mc == 0), stop=(mc == NT - 1))
                    nc.tensor.matmul(cps1[:], vh[:, mc * P:(mc + 1) * P],
                                     aT_mc[:, 512:1024],
                                     start=(mc == 0), stop=(mc == NT - 1))
                nc.vector.tensor_copy(cT[:, 0:512], cps0[:])
                nc.vector.tensor_copy(cT[:, 512:1024], cps1[:])
                nc.sync.dma_start(out=cx_d[h].bitcast(f32r), in_=cT[:])

        # ---------------- Phase 3: output projection ----------------
        with ExitStack() as ctx:
            wop = ctx.enter_context(tc.tile_pool(name="wop", bufs=H))
            ctip = ctx.enter_context(tc.tile_pool(name="cti", bufs=3))
            oacc = ctx.enter_context(tc.tile_pool(name="oacc", bufs=3))
            ops = ctx.enter_context(tc.tile_pool(name="ops", bufs=4, space="PSUM"))
            wo_sb = []
            for h in range(H):
                t = wop.tile([P, C], f32r, tag="wo")
                nc.sync.dma_start(out=t[:],
                                  in_=w_d["o"][h * P:(h + 1) * P, :].bitcast(f32r))
                wo_sb.append(t)
            for i in range(NT):
                cti = ctip.tile([P, N], f32r, tag="cti")
                nc.sync.dma_start(
                    out=cti[:].rearrange("p (h n) -> p h n", h=H),
                    in_=cx_d[:, :, i * P:(i + 1) * P]
                    .rearrange("h p n -> p h n").bitcast(f32r))
                oa = oacc.tile([P, C], f32, tag="oa")
                for half in range(2):
                    ps = ops.tile([P, C // 2], f32, tag="op")
                    for h in range(H):
                        nc.tensor.matmul(ps[:],
                                         cti[:, h * P:(h + 1) * P],
                                         wo_sb[h][:, half * 512:(half + 1) * 512],
                                         start=(h == 0), stop=False)
                    nc.tensor.matmul(ps[:], ones_sb[:],
                                     bias_rows[:, C + half * 512:C + (half + 1) * 512],
                                     start=False, stop=True)
                    nc.scalar.copy(oa[:, half * 512:(half + 1) * 512], ps[:])
                nc.sync.dma_start(out=out_d[i * P:(i + 1) * P, :], in_=oa[:])

    nc.compile()
    return nc


def get_nc():
    if "nc" not in _CACHE:
        _CACHE["nc"] = _build()
    return _CACHE["nc"]


def kernel(x, Wq, bq, Wk, bk, Wv, bv, Wo, bo):
    nc = get_nc()
    shared = {"Wq": np.ascontiguousarray(Wq, dtype=np.float32),
              "bq": np.ascontiguousarray(bq, dtype=np.float32),
              "Wk": np.ascontiguousarray(Wk, dtype=np.float32),
              "bk": np.ascontiguousarray(bk, dtype=np.float32),
              "Wv": np.ascontiguousarray(Wv, dtype=np.float32),
              "bv": np.ascontiguousarray(bv, dtype=np.float32),
              "Wo": np.ascontiguousarray(Wo, dtype=np.float32),
              "bo": np.ascontiguousarray(bo, dtype=np.float32)}
    in_maps = [{"x": np.ascontiguousarray(np.asarray(x)[b], dtype=np.float32),
                **shared} for b in range(B)]
    res = run_bass_kernel_spmd(nc, in_maps, core_ids=list(range(B)))
    out = np.stack([res.results[b]["out"] for b in range(B)])
    attn = np.stack([res.results[b]["attn"] for b in range(B)])
    return out, attn


# revision 10
# speedup vs baseline: 1.1206x; 1.1206x over previous
"""EntmaxAttention TRN2 kernel.

Full inputs -> full outputs. Internally shards the batch (B=8) across 8
NeuronCores (data parallel, no collectives). Each core runs the complete
attention pipeline for its batch element:

  Q^T/K^T/V projections (fp32r matmuls) -> per-head scores (fp32r) ->
  entmax-1.5 via 5 support-set quadratic-solve iterations (exact solve of
  sum(relu(x-tau)^2)=1 on the current support; converges to the same fp32
  fixpoint as the reference's 50-step bisection) -> attn normalize ->
  PE-transpose attn -> ctx^T (fp32r) -> output projection (fp32r).

Engine split per entmax iteration (per [128,1024] tile):
  DVE : w = max(xm, tau)  (accum -> sum w  => m1 = sum w - 1024*tau)
  ACT : p = (tau - w)^2   (accum -> m2)
  DVE : count = sum(xm > tau)
Per-row state for all 8 tiles of a head lives in one [128, *] tile so the
scalar updates are single instructions per head-iteration.
"""

import sys
import numpy as np
from contextlib import ExitStack

if "/opt/trn_rl_repo" not in sys.path:
    sys.path.insert(0, "/opt/trn_rl_repo")

import concourse.bacc as bacc
import concourse.tile as tile
import concourse.mybir as mybir
from concourse.bass_utils import run_bass_kernel_spmd
from concourse.masks import make_identity

f32 = mybir.dt.float32
f32r = mybir.dt.float32r
Alu = mybir.AluOpType
Act = mybir.ActivationFunctionType
AxX = mybir.AxisListType.X

B, N, C, H = 8, 1024, 1024, 8
D = C // H          # 128
P = 128             # partitions
NT = N // P         # 8 row tiles per head
CH = C // P         # 8 contraction chunks
SCALE = float(np.float32(0.5 / np.sqrt(D)))   # (alpha-1) / sqrt(D)
N_ITERS = 5

_CACHE = {}


def _build():
    nc = bacc.Bacc("TRN2", target_bir_lowering=False, debug=False,
                   enable_asserts=False, num_devices=8)

    x_d = nc.dram_tensor("x", [N, C], f32, kind="ExternalInput").ap()
    w_d = {}
    b_d = {}
    for nm in ("q", "k", "v", "o"):
        w_d[nm] = nc.dram_tensor(f"W{nm}", [C, C], f32, kind="ExternalInput").ap()
        b_d[nm] = nc.dram_tensor(f"b{nm}", [C], f32, kind="ExternalInput").ap()
    out_d = nc.dram_tensor("out", [N, C], f32, kind="ExternalOutput").ap()
    attn_d = nc.dram_tensor("attn", [H, N, N], f32, kind="ExternalOutput").ap()

    # DRAM scratch: Q^T/K^T (chunk == head), V (natural), ctx^T per head
    qT_d = nc.dram_tensor("qT_scr", [H, P, N], f32).ap()
    kT_d = nc.dram_tensor("kT_scr", [H, P, N], f32).ap()
    v_d = nc.dram_tensor("v_scr", [N, C], f32).ap()
    cx_d = nc.dram_tensor("cx_scr", [H, P, N], f32).ap()

    ones_host = nc.inline_tensor(np.ones((1, P), dtype=np.float32), name="ones_c")

    with tile.TileContext(nc) as tc, ExitStack() as octx:
        consts = octx.enter_context(tc.tile_pool(name="consts", bufs=1))
        ident = consts.tile([P, P], f32)
        make_identity(nc, ident[:])
        ones_sb = consts.tile([1, P], f32r)
        nc.sync.dma_start(out=ones_sb[:], in_=ones_host.ap().bitcast(f32r))
        # bias rows ([1, C]) for the ones-trick matmuls (bv, bo)
        bias_rows = consts.tile([1, 2 * C], f32r)
        nc.sync.dma_start(out=bias_rows[:, 0:C],
                          in_=b_d["v"].rearrange("(a c) -> a c", a=1).bitcast(f32r))
        nc.sync.dma_start(out=bias_rows[:, C:2 * C],
                          in_=b_d["o"].rearrange("(a c) -> a c", a=1).bitcast(f32r))
        # bias columns ([P, CH]) for Q^T / K^T evictions
        bias_cols = consts.tile([P, 2 * CH], f32)
        nc.sync.dma_start(out=bias_cols[:, 0:CH],
                          in_=b_d["q"].rearrange("(c p) -> p c", p=P))
        nc.sync.dma_start(out=bias_cols[:, CH:2 * CH],
                          in_=b_d["k"].rearrange("(c p) -> p c", p=P))

        # ---------------- Phase 0+1: x^T, projections ----------------
        with ExitStack() as ctx:
            xpool = ctx.enter_context(tc.tile_pool(name="xsb", bufs=NT))
            xTpool = ctx.enter_context(tc.tile_pool(name="xT", bufs=CH))
            tps = ctx.enter_context(tc.tile_pool(name="tps", bufs=4, space="PSUM"))
            x_sb = []
            for i in range(NT):
                t = xpool.tile([P, C], f32, tag="xsb")
                nc.sync.dma_start(out=t[:], in_=x_d[i * P:(i + 1) * P, :])
                x_sb.append(t)
            xT = [xTpool.tile([P, N], f32r, tag="xT", name=f"xT{ck}")
                  for ck in range(CH)]
            for ck in range(CH):
                for i2 in range(0, NT, 4):
                    pt = tps.tile([P, 4 * P], f32, tag="tp")
                    for j in range(4):
                        i = i2 + j
                        nc.tensor.transpose(pt[:, j * P:(j + 1) * P],
                                            x_sb[i][:, ck * P:(ck + 1) * P],
                                            ident[:])
                    nc.vector.tensor_copy(xT[ck][:, i2 * P:(i2 + 4) * P], pt[:])

            wpool = ctx.enter_context(tc.tile_pool(name="wsb", bufs=2 * CH))
            stage = ctx.enter_context(tc.tile_pool(name="stage", bufs=3))
            pps = ctx.enter_context(tc.tile_pool(name="pps", bufs=4, space="PSUM"))

            for wi, nm in enumerate(("q", "k")):
                w_sb = []
                for ck in range(CH):
                    t = wpool.tile([P, C], f32r, tag="w")
                    nc.sync.dma_start(out=t[:],
                                      in_=w_d[nm][ck * P:(ck + 1) * P, :].bitcast(f32r))
                    w_sb.append(t)
                dst = qT_d if nm == "q" else kT_d
                bcol = bias_cols[:, wi * CH:(wi + 1) * CH]
                for cp in range(CH):
                    acc = stage.tile([P, N], f32, tag="stage")
                    for half in range(2):
                        ps = pps.tile([P, N // 2], f32, tag="pp")
                        for ck in range(CH):
                            nc.tensor.matmul(
                                ps[:],
                                w_sb[ck][:, cp * P:(cp + 1) * P],
                                xT[ck][:, half * 512:(half + 1) * 512],
                                start=(ck == 0), stop=(ck == CH - 1))
                        nc.scalar.activation(acc[:, half * 512:(half + 1) * 512],
                                             ps[:], Act.Identity,
                                             bias=bcol[:, cp:cp + 1], scale=1.0)
                    nc.sync.dma_start(out=dst[cp], in_=acc[:])

            # V = x @ Wv + bv  (natural layout, row chunks)
            w_sb = []
            for ck in range(CH):
                t = wpool.tile([P, C], f32r, tag="w")
                nc.sync.dma_start(out=t[:],
                                  in_=w_d["v"][ck * P:(ck + 1) * P, :].bitcast(f32r))
                w_sb.append(t)
            for mc in range(NT):
                acc = stage.tile([P, C], f32, tag="stage")
                for half in range(2):
                    ps = pps.tile([P, C // 2], f32, tag="pp")
                    for ck in range(CH):
                        nc.tensor.matmul(
                            ps[:],
                            xT[ck][:, mc * P:(mc + 1) * P],
                            w_sb[ck][:, half * 512:(half + 1) * 512],
                            start=(ck == 0), stop=False)
                    nc.tensor.matmul(ps[:], ones_sb[:],
                                     bias_rows[:, half * 512:(half + 1) * 512],
                                     start=False, stop=True)
                    nc.scalar.copy(acc[:, half * 512:(half + 1) * 512], ps[:])
                nc.sync.dma_start(out=v_d[mc * P:(mc + 1) * P, :], in_=acc[:])

        # ---------------- Phase 2: per-head attention ----------------
        with ExitStack() as ctx:
            qkv = ctx.enter_context(tc.tile_pool(name="qkv", bufs=2))
            xmp = ctx.enter_context(tc.tile_pool(name="xm", bufs=10))
            wsc = ctx.enter_context(tc.tile_pool(name="wsc", bufs=4))
            ppool = ctx.enter_context(tc.tile_pool(name="pp", bufs=5))
            apool = ctx.enter_context(tc.tile_pool(name="attn", bufs=NT + 1))
            aT = ctx.enter_context(tc.tile_pool(name="aT", bufs=3))
            stp = ctx.enter_context(tc.tile_pool(name="state", bufs=6))
            cxp = ctx.enter_context(tc.tile_pool(name="cxp", bufs=2))
            sps = ctx.enter_context(tc.tile_pool(name="sps", bufs=2, space="PSUM"))
            tps2 = ctx.enter_context(tc.tile_pool(name="tps2", bufs=2, space="PSUM"))
            cps = ctx.enter_context(tc.tile_pool(name="cps", bufs=2, space="PSUM"))

            G = 4  # tiles per entmax group (half a head)

            def group_state(h, half):
                st = stp.tile([P, 16 * G], f32, tag="st", name=f"st{h}_{half}")
                names = ("mx", "tau", "sw", "m2", "kc", "m1", "t2", "m1s",
                         "disc", "sq", "num", "rk", "dlt", "sS", "rS")
                return {nm: st[:, i * G:(i + 1) * G] for i, nm in enumerate(names)}

            def scores_group(h, half, qh, kh, S):
                xm = []
                for g in range(G):
                    nt = half * G + g
                    t = xmp.tile([P, N], f32, tag="xm", name=f"xm{h}_{nt}")
                    ps = sps.tile([P, N], f32, tag="sp")
                    for hf in range(2):
                        nc.tensor.matmul(ps[:, hf * 512:(hf + 1) * 512],
                                         qh[:, nt * P:(nt + 1) * P],
                                         kh[:, hf * 512:(hf + 1) * 512],
                                         start=True, stop=True)
                    # fused evict: xm = SCALE*psum, accum-max -> mx
                    nc.vector.tensor_scalar(
                        out=t[:], in0=ps[:], scalar1=SCALE, scalar2=None,
                        op0=Alu.mult, op1=Alu.max, accum_out=S["mx"][:, g:g + 1])
                    xm.append(t)
                nc.vector.tensor_scalar(out=S["tau"], in0=S["mx"], scalar1=-1.0,
                                        scalar2=None, op0=Alu.add)
                return xm

            def iter_passes(h, half, it, xm, S, at, last, at_n):
                tau = S["tau"]
                for g in range(G):
                    w = wsc.tile([P, N], f32, tag="w", name=f"w{h}_{half}_{g}")
                    nc.vector.tensor_scalar(
                        out=w[:], in0=xm[g][:], scalar1=tau[:, g:g + 1],
                        scalar2=None, op0=Alu.max, op1=Alu.add,
                        accum_out=S["sw"][:, g:g + 1])
                    if last:
                        pt = ppool.tile([P, N], f32, tag="p",
                                        name=f"p{h}_{half}_{g}")
                        acc = S["sS"][:, g:g + 1]
                    else:
                        pt = wsc.tile([P, N], f32, tag="psc",
                                      name=f"psc{h}_{half}_{g}")
                        acc = S["m2"][:, g:g + 1]
                    nc.scalar.activation(pt[:], w[:], Act.Square,
                                         bias=tau[:, g:g + 1], scale=-1.0,
                                         accum_out=acc)
                    if last:
                        at.append(pt)
                        if g == G - 1:
                            nc.vector.reciprocal(S["rS"], S["sS"])
                            for g2 in range(G):
                                nt = half * G + g2
                                a = apool.tile([P, N], f32, tag="a",
                                               name=f"a{h}_{nt}")
                                nc.scalar.activation(
                                    a[:], at[half * G + g2][:], Act.Copy,
                                    bias=0.0, scale=S["rS"][:, g2:g2 + 1])
                                nc.sync.dma_start(
                                    out=attn_d[h, nt * P:(nt + 1) * P, :],
                                    in_=a[:])
                                at_n.append(a)
                    else:
                        msk = wsc.tile([P, N], f32, tag="psc",
                                       name=f"msk{h}_{half}_{g}")
                        nc.vector.tensor_scalar(
                            out=msk[:], in0=xm[g][:],
                            scalar1=tau[:, g:g + 1], scalar2=None,
                            op0=Alu.is_gt, op1=Alu.add,
                            accum_out=S["kc"][:, g:g + 1])

            def chain_dve(S):
                # tau += (m1 - sqrt(max(m1^2 - kc*(m2-1), 0))) / kc   [DVE part 1]
                nc.vector.scalar_tensor_tensor(out=S["m1"], in0=S["tau"],
                                               scalar=-float(N), in1=S["sw"],
                                               op0=Alu.mult, op1=Alu.add)
                nc.vector.scalar_tensor_tensor(out=S["t2"], in0=S["m2"], scalar=-1.0,
                                               in1=S["kc"], op0=Alu.add, op1=Alu.mult)
                nc.vector.reciprocal(S["rk"], S["kc"])

            def chain_act1(S):
                nc.scalar.activation(S["m1s"], S["m1"], Act.Square, bias=0.0, scale=1.0)

            def chain_dve2(S):
                nc.vector.scalar_tensor_tensor(out=S["disc"], in0=S["t2"], scalar=-1.0,
                                               in1=S["m1s"], op0=Alu.mult, op1=Alu.add)
                nc.vector.tensor_scalar(out=S["disc"], in0=S["disc"], scalar1=0.0,
                                        scalar2=None, op0=Alu.max)

            def chain_act2(S):
                nc.scalar.activation(S["sq"], S["disc"], Act.Sqrt, bias=0.0, scale=1.0)

            def chain_dve3(S):
                nc.vector.scalar_tensor_tensor(out=S["num"], in0=S["sq"], scalar=-1.0,
                                               in1=S["m1"], op0=Alu.mult, op1=Alu.add)
                nc.vector.scalar_tensor_tensor(out=S["dlt"], in0=S["num"], scalar=1.0,
                                               in1=S["rk"], op0=Alu.mult, op1=Alu.mult)
                nc.vector.scalar_tensor_tensor(out=S["tau"], in0=S["dlt"], scalar=1.0,
                                               in1=S["tau"], op0=Alu.mult, op1=Alu.add)

            for h in range(H):
                qh = qkv.tile([P, N], f32r, tag="qh", name=f"qh{h}")
                kh = qkv.tile([P, N], f32r, tag="kh", name=f"kh{h}")
                vh = qkv.tile([P, C], f32r, tag="vh", name=f"vh{h}")
                nc.sync.dma_start(out=qh[:], in_=qT_d[h].bitcast(f32r))
                nc.sync.dma_start(out=kh[:], in_=kT_d[h].bitcast(f32r))
                nc.sync.dma_start(
                    out=vh[:].rearrange("p (c d) -> p c d", c=CH),
                    in_=v_d[:, h * D:(h + 1) * D]
                    .rearrange("(c p) d -> p c d", p=P).bitcast(f32r))

                S0 = group_state(h, 0)
                S1 = group_state(h, 1)
                xm0 = scores_group(h, 0, qh, kh, S0)
                xm1 = scores_group(h, 1, qh, kh, S1)
                at = []
                at_n = []
                for it in range(N_ITERS):
                    last = (it == N_ITERS - 1)
                    iter_passes(h, 0, it, xm0, S0, at, last, at_n)
                    iter_passes(h, 1, it, xm1, S1, at, last, at_n)
                    if last:
                        break
                    chain_dve(S0); chain_dve(S1)
                    chain_act1(S0); chain_act1(S1)
                    chain_dve2(S0); chain_dve2(S1)
                    chain_act2(S0); chain_act2(S1)
                    chain_dve3(S0); chain_dve3(S1)

                at = at_n

                # transpose attn -> attnT chunks; ctx^T accumulation
                cT = cxp.tile([P, N], f32r, tag="cT", name=f"cT{h}")
                cps0 = cps.tile([P, N // 2], f32, tag="cp", name=f"cps0_{h}")
                cps1 = cps.tile([P, N // 2], f32, tag="cp", name=f"cps1_{h}")
                for mc in range(NT):
                    aT_mc = aT.tile([P, N], f32r, tag="aT", name=f"aT{h}_{mc}")
                    for nt2 in range(0, NT, 4):
                        pt = tps2.tile([P, 4 * P], f32, tag="tp2")
                        for j in range(4):
                            nt = nt2 + j
                            nc.tensor.transpose(pt[:, j * P:(j + 1) * P],
                                                at[nt][:, mc * P:(mc + 1) * P],
                                                ident[:])
                        if (nt2 // 4) % 2 == 0:
                            nc.vector.tensor_copy(aT_mc[:, nt2 * P:(nt2 + 4) * P],
                                                  pt[:])
                        else:
                            nc.scalar.copy(aT_mc[:, nt2 * P:(nt2 + 4) * P], pt[:])
                    nc.tensor.matmul(cps0[:], vh[:, mc * P:(mc + 1) * P],
                                     aT_mc[:, 0:512],
                                     start=(mc == 0), stop=(mc == NT - 1))
                    nc.tensor.matmul(cps1[:], vh[:, mc * P:(mc + 1) * P],
                                     aT_mc[:, 512:1024],
                                     start=(mc == 0), stop=(mc == NT - 1))
                nc.vector.tensor_copy(cT[:, 0:512], cps0[:])
                nc.vector.tensor_copy(cT[:, 512:1024], cps1[:])
                nc.sync.dma_start(out=cx_d[h].bitcast(f32r), in_=cT[:])

        # ---------------- Phase 3: output projection ----------------
        with ExitStack() as ctx:
            wop = ctx.enter_context(tc.tile_pool(name="wop", bufs=H))
            ctip = ctx.enter_context(tc.tile_pool(name="cti", bufs=3))
            oacc = ctx.enter_context(tc.tile_pool(name="oacc", bufs=3))
            ops = ctx.enter_context(tc.tile_pool(name="ops", bufs=4, space="PSUM"))
            wo_sb = []
            for h in range(H):
                t = wop.tile([P, C], f32r, tag="wo")
                nc.sync.dma_start(out=t[:],
                                  in_=w_d["o"][h * P:(h + 1) * P, :].bitcast(f32r))
                wo_sb.append(t)
            for i in range(NT):
                cti = ctip.tile([P, N], f32r, tag="cti")
                nc.sync.dma_start(
                    out=cti[:].rearrange("p (h n) -> p h n", h=H),
                    in_=cx_d[:, :, i * P:(i + 1) * P]
                    .rearrange("h p n -> p h n").bitcast(f32r))
                oa = oacc.tile([P, C], f32, tag="oa")
                for half in range(2):
                    ps = ops.tile([P, C // 2], f32, tag="op")
                    for h in range(H):
                        nc.tensor.matmul(ps[:],
                                         cti[:, h * P:(h + 1) * P],
                                         wo_sb[h][:, half * 512:(half + 1) * 512],
                                         start=(h == 0), stop=False)
                    nc.tensor.matmul(ps[:], ones_sb[:],
                                     bias_rows[:, C + half * 512:C + (half + 1) * 512],
                                     start=False, stop=True)
                    nc.scalar.copy(oa[:, half * 512:(half + 1) * 512], ps[:])
                nc.sync.dma_start(out=out_d[i * P:(i + 1) * P, :], in_=oa[:])

    nc.compile()
    return nc


def get_nc():
    if "nc" not in _CACHE:
        _CACHE["nc"] = _build()
    return _CACHE["nc"]


def kernel(x, Wq, bq, Wk, bk, Wv, bv, Wo, bo):
    nc = get_nc()
    shared = {"Wq": np.ascontiguousarray(Wq, dtype=np.float32),
              "bq": np.ascontiguousarray(bq, dtype=np.float32),
              "Wk": np.ascontiguousarray(Wk, dtype=np.float32),
              "bk": np.ascontiguousarray(bk, dtype=np.float32),
              "Wv": np.ascontiguousarray(Wv, dtype=np.float32),
              "bv": np.ascontiguousarray(bv, dtype=np.float32),
              "Wo": np.ascontiguousarray(Wo, dtype=np.float32),
              "bo": np.ascontiguousarray(bo, dtype=np.float32)}
    in_maps = [{"x": np.ascontiguousarray(np.asarray(x)[b], dtype=np.float32),
                **shared} for b in range(B)]
    res = run_bass_kernel_spmd(nc, in_maps, core_ids=list(range(B)))
    out = np.stack([res.results[b]["out"] for b in range(B)])
    attn = np.stack([res.results[b]["attn"] for b in range(B)])
    return out, attn
